# revision 57
# baseline (speedup 1.0000x reference)
"""CTC loss (keras ctc_batch_cost semantics) on 8 Trainium2 NeuronCores.

Data-parallel over batch: 1024 samples -> 8 cores x 128 samples
(one sample per SBUF partition).  Host prep is integer-only (gather
index tables + skip masks); all float work runs on device.

Device pipeline (per core, per 128-step T-half):
  A. load y_pred [tau,(b,c)] 16-sample group tiles, ACT-cast fp32->bf16
     with +EPS, PE-transpose each sample's [tau,c] square to [c,tau] in
     PSUM (identity matmul, 2 banks/group), batch-copy PSUM->SBUF
     (Vector while idle, else Scalar), one 512KB DMA store of bf16 rows
     to an HBM scratch.
  B. dma_gather: rows (b, c=label_j) -> ptil[b, (blk, tau)] via int16
     row tables (8 SWDGE gathers across 4 queues); the blank block is a
     plain strided DMA (same class-127 row for every sample).
  C. per-column scale: gmax = max over gather-0's 8 blocks + blank
     (Vector), ginv = Exp(-Ln(g)+RHAT) on Scalar (exact value cancels
     on host); blank + first batch scaled on Vector so scan s=0 starts
     immediately, remaining 4-block batches on GpSimd chase the
     gathers.  The h1 instance is emitted mid-k0 in the DVE stream.
  D. s-sweep over the 129-row extended CTC lattice: each row's
     recursion v_t = (e_t + v_{t-1}) * p_t is ONE tensor_tensor_scan
     along the free dim (fp32 internal state, bf16-stored treg); row
     coupling e_t = v^{s-1}_{t-1} + m*v^{s-2}_{t-1} is one
     scalar_tensor_tensor (odd rows) or a shifted view (even).
     Probability domain, per-half max-renorm keeps fp32 range.
Host assembles loss = -(log lsum + sum log bmax - sum log ginv) in f64.
"""
from contextlib import ExitStack

import numpy as np
import ml_dtypes

import concourse.bass as bass
import concourse.tile as tile
from concourse import bacc, mybir
from concourse.bass_utils import run_bass_kernel_spmd
from concourse.masks import make_identity

F32 = mybir.dt.float32
BF16 = mybir.dt.bfloat16
I32 = mybir.dt.int32
I16 = mybir.dt.int16
AF = mybir.ActivationFunctionType
ALU = mybir.AluOpType

B, T, C, L = 1024, 256, 128, 64
S = 2 * L + 1          # 129 extended states
NBLK = L + 1           # 64 label blocks + 1 blank block
BLANK = C - 1
EPS = 1e-7
RHAT = 0.4             # per-step prob boost exp(RHAT) centers chunk decay
TC = 128               # scan chunk length == tau-half
NCH = T // TC          # 2
W = T + 1              # Treg slot width: col0 = v_{-1}, col 1+t = v_t
SLOTS = S + 2          # 2 permanent zero rows + 129 state rows
PB = 128               # samples per core
NCORES = 8
SGRP = 16              # samples per load/cast/transpose group
NGRP = PB // SGRP      # 8 groups per half
MBATCH = 4             # label blocks per ginv-scale mul
GSPANS = [(2 * i, 2 * i + 2) for i in range(4)] + \
         [(8 * i, 8 * i + 8) for i in range(1, 8)]


def _host_prep(y_true_shard: np.ndarray):
    yt = y_true_shard.astype(np.int64)
    # scratch row ids: row(b, c, h) = (b*C + c)*2 + h, 256B each
    spans = [(2 * i, 2 * i + 2) for i in range(4)] + \
        [(8 * i, 8 * i + 8) for i in range(1, 8)]
    tabs = []
    for h in range(NCH):
        base = (np.arange(PB) * C) * 2 + h
        for bl0, bl1 in spans:
            nb = bl1 - bl0
            idx_flat = np.empty(nb * PB, np.int32)
            for jl in range(nb):
                idx_flat[jl * PB:(jl + 1) * PB] = base + yt[:, bl0 + jl] * 2
            tabs.append(idx_flat.reshape(nb * PB // 16, 16).T)
    table16 = np.concatenate(tabs, axis=1)                   # [16, 1024]
    idxs = np.tile(table16, (8, 1)).astype(np.int16)         # [128, 1024]
    m01 = np.ones((PB, L), np.float32)
    m01[:, 1:] = (yt[:, 1:] != yt[:, :-1]).astype(np.float32)
    m01[:, 0] = 0.0
    return {"idxs": idxs, "m01": m01}


def _emit(ctx: ExitStack, tc: tile.TileContext, y_in, idxs_in, m01_in,
          raw_out, ginv_out):
    nc = tc.nc

    persist = ctx.enter_context(tc.tile_pool(name="persist", bufs=1))
    stage = ctx.enter_context(tc.tile_pool(name="stage", bufs=4))
    cpool = ctx.enter_context(tc.tile_pool(name="cbuf", bufs=4))
    scratch = ctx.enter_context(tc.tile_pool(name="scratch", bufs=2))
    psum = ctx.enter_context(tc.tile_pool(name="ps", bufs=4, space="PSUM"))
    dram = ctx.enter_context(tc.tile_pool(name="dram", bufs=1, space="DRAM"))

    idxs = persist.tile([PB, 2 * L * PB // 16], I16)
    nc.sync.dma_start(idxs[:], idxs_in[:])
    m01 = persist.tile([PB, L], F32)
    nc.sync.dma_start(m01[:], m01_in[:])

    ident = persist.tile([TC, TC], BF16)
    make_identity(nc, ident[:])



    treg_t = persist.tile([PB, SLOTS * W], BF16)
    # zero rows: the 2 permanent v==0 slots, plus column 0 of every state
    # row (t=0 coupling reads); everything else is scan-written.
    nc.gpsimd.memset(treg_t[:, 0:2 * W], 0.0)
    nc.gpsimd.memset(treg_t[:, 2 * W:SLOTS * W:W], 0.0)
    raw = persist.tile([PB, NCH], F32)
    epsb = persist.tile([PB, 1], F32)
    nc.vector.memset(epsb[:], EPS)
    rhatb = persist.tile([PB, 1], F32)
    nc.vector.memset(rhatb[:], RHAT)

    # HBM scratch: row (b, c, h) = TC bf16 = 256B
    ytT = dram.tile([PB * C * NCH, TC], BF16)
    ytT4 = ytT[:].rearrange("(b c h) t -> b c h t", b=PB, c=C, h=NCH)

    # SWDGE warmup: burn the first descriptor-gen latency on a dummy gather
    # at t~0, fully off the critical path (own dram scratch + iota indices).
    wdram = dram.tile([PB, TC], BF16)
    warm = scratch.tile([PB, TC], BF16, tag="warm")
    warmidx = scratch.tile([PB, 8], I16, tag="warmidx")
    nc.gpsimd.iota(warmidx[:], pattern=[[1, 8]], base=0, channel_multiplier=0)
    nc.gpsimd.dma_gather(
        warm[:].rearrange("p (i e) -> p i e", i=1),
        wdram[:], warmidx[:],
        num_idxs=PB, num_idxs_reg=PB, elem_size=TC, queue_num=0)



    ptil, ginvb = [], []
    for h in range(NCH):
        ptil_h = persist.tile([PB, NBLK * TC], BF16, tag=f"ptil{h}")
        ginvb_h = persist.tile([PB, TC], BF16, tag=f"ginvb{h}")
        ptil.append(ptil_h); ginvb.append(ginvb_h)

    def phase_abc(h):
        # A: load + cast(+eps) + PE transpose + batched store of bf16 rows.
        # h0 group 0 is split into 4-sample sub-loads across both queues so
        # the cast/transpose pipeline starts ~6us earlier.
        for g in range(NGRP):
            b0 = g * SGRP
            ld = stage.tile([PB, SGRP * C], F32, tag="ld")
            bf = stage.tile([PB, SGRP * C], BF16, tag="bf")
            nsub = 4 if (h == 0 and g == 0) else 1
            sw = SGRP // nsub
            for sub in range(nsub):
                s0 = sub * sw
                eng_ld = nc.sync if ((g + sub) % 2 == 0) else nc.scalar
                eng_ld.dma_start(
                    ld[:, s0 * C:(s0 + sw) * C]
                    .rearrange("p (b c) -> p b c", b=sw),
                    y_in[b0 + s0:b0 + s0 + sw, h * TC:(h + 1) * TC, :]
                    .rearrange("b t c -> t b c"))
                nc.scalar.activation(bf[:, s0 * C:(s0 + sw) * C],
                                     ld[:, s0 * C:(s0 + sw) * C],
                                     AF.Identity, bias=epsb[:, 0:1])
            ps = psum.tile([C, SGRP * TC], BF16, tag="ps")
            for i in range(SGRP):
                nc.tensor.transpose(ps[:, i * TC:(i + 1) * TC],
                                    bf[:, i * C:(i + 1) * C], ident[:])
            cp = stage.tile([C, SGRP * TC], BF16, tag="cp")
            # Vector is idle during the h0 prologue; keep it clear of
            # copies once the scan chain is running (h1).
            if h == 0 and g % 2 == 1:
                nc.vector.tensor_copy(cp[:], ps[:])
            else:
                nc.scalar.copy(cp[:], ps[:])
            eng_st = nc.scalar if (g % 2 == 0) else nc.sync
            eng_st.dma_start(
                ytT4[b0:b0 + SGRP, :, h, :].rearrange("b c t -> c b t"),
                cp[:].rearrange("p (b t) -> p b t", t=TC))
        # B: label gathers; the ginv-critical blocks 0-7 go as two parallel
        # 512-idx gathers (halves the first descriptor-gen latency when the
        # per-queue gen processors run concurrently), bulk in 8-block groups.
        col = h * 512
        for gi, (bl0, bl1) in enumerate(GSPANS):
            ncols = (bl1 - bl0) * PB // 16
            nc.gpsimd.dma_gather(
                ptil[h][:, bl0 * TC:bl1 * TC]
                .rearrange("p (i e) -> p i e", e=TC),
                ytT[:],
                idxs[:, col:col + ncols],
                num_idxs=(bl1 - bl0) * PB, num_idxs_reg=(bl1 - bl0) * PB,
                elem_size=TC, queue_num=gi % 4)
            col += ncols
        nc.sync.dma_start(ptil[h][:, L * TC:NBLK * TC], ytT4[:, BLANK, h, :])

    gmaxs = []
    for h in range(NCH):
        gmax_h = scratch.tile([PB, TC], F32, tag=f"gmax{h}")
        gmaxs.append(gmax_h)

    def phase_c_vec(h):
        # gmax over gather-0's 8 blocks + blank, on Vector (Pool has no max).
        nc.vector.tensor_reduce(
            gmaxs[h][:],
            ptil[h][:, 0:8 * TC].rearrange("p (blk t) -> p t blk", blk=8),
            axis=mybir.AxisListType.X, op=ALU.max)
        nc.vector.tensor_max(gmaxs[h][:], gmaxs[h][:],
                             ptil[h][:, L * TC:NBLK * TC])
        ginv32 = scratch.tile([PB, TC], F32, tag=f"ginv32{h}")
        nc.vector.reciprocal(ginv32[:], gmaxs[h][:])
        nc.vector.tensor_scalar_mul(ginvb[h][:], ginv32[:],
                                    float(np.exp(RHAT)))

    def phase_c_rest(h):
        # ginv computed on Vector in phase_c_vec (no ACT-table swaps or
        # cross-engine hop on the critical chain); scale muls below, blank
        # block first so scan s=0 starts immediately.
        nc.scalar.dma_start(ginv_out[:, h * TC:(h + 1) * TC], ginvb[h][:])
        gv = ginvb[h][:, None, :].broadcast_to([PB, MBATCH, TC])
        blank_sl = ptil[h][:, L * TC:NBLK * TC]
        # blank + first batch on Vector: scan s=0 starts right after ginv
        # lands, skipping the GpSimd semaphore round-trip.
        nc.vector.tensor_mul(blank_sl, blank_sl, ginvb[h][:])
        sl0 = (ptil[h][:, 0:MBATCH * TC]
               .rearrange("p (r t) -> p r t", r=MBATCH))
        nc.vector.tensor_mul(sl0, sl0, gv)
        for blk in range(MBATCH, L, MBATCH):
            sl = (ptil[h][:, blk * TC:(blk + MBATCH) * TC]
                  .rearrange("p (r t) -> p r t", r=MBATCH))
            nc.gpsimd.tensor_mul(sl, sl, gv)

    def sb(s):  # Treg slot base col
        return (s + 2) * W

    def phase_d(k, mid=None):
        t0 = k * TC
        if k > 0:
            start = 2 * W + t0
            bcols = treg_t[:, start:start + (S - 1) * W + 1:W]
            nc.vector.tensor_reduce(raw[:, k:k + 1], bcols,
                                    axis=mybir.AxisListType.X, op=ALU.max)
            rinv = scratch.tile([PB, 1], F32, tag="rinv")
            nc.vector.reciprocal(rinv[:], raw[:, k:k + 1])
            nc.vector.tensor_scalar_mul(bcols, bcols, rinv[:])
        for s in range(S):
            if mid is not None and s == 100:
                mid()
            base = sb(s)
            if s % 2 == 1:
                j = (s - 1) // 2
                c = cpool.tile([PB, TC], F32, tag="c")
                nc.vector.scalar_tensor_tensor(
                    c[:],
                    treg_t[:, sb(s - 2) + t0: sb(s - 2) + t0 + TC],
                    m01[:, j:j + 1],
                    treg_t[:, sb(s - 1) + t0: sb(s - 1) + t0 + TC],
                    op0=ALU.mult, op1=ALU.add,
                )
                d0 = c[:]
                blk = j
            else:
                d0 = treg_t[:, sb(s - 1) + t0: sb(s - 1) + t0 + TC]
                blk = L
            # chunk 0: immediate initial (col-0 cells stay 0 — they feed the
            # t=0 coupling reads of rows s+1, s+2)
            if k == 0:
                initial = 1.0 if s <= 1 else 0.0
            else:
                initial = treg_t[:, base + t0: base + t0 + 1]
            nc.vector.tensor_tensor_scan(
                treg_t[:, base + 1 + t0: base + 1 + t0 + TC],
                d0,
                ptil[k][:, blk * TC:(blk + 1) * TC],
                initial,
                op0=ALU.add, op1=ALU.mult,
            )

    phase_abc(0)
    phase_c_vec(0)
    phase_c_rest(0)
    phase_abc(1)
    phase_d(0, mid=lambda: (phase_c_vec(1), phase_c_rest(1)))
    phase_d(1)

    b127 = sb(127) + T
    b128 = sb(128) + T
    nc.vector.tensor_add(raw[:, 0:1], treg_t[:, b127:b127 + 1],
                         treg_t[:, b128:b128 + 1])
    nc.sync.dma_start(raw_out[:], raw[:])


_CACHE: dict = {}


def _build():
    nc = bacc.Bacc("TRN2", target_bir_lowering=False, debug=False,
                   num_devices=NCORES, num_swdge_queues=4)
    y_in = nc.dram_tensor("ypred", [PB, T, C], F32, kind="ExternalInput").ap()
    idxs_in = nc.dram_tensor("idxs", [PB, 2 * L * PB // 16], I16,
                             kind="ExternalInput").ap()
    m01_in = nc.dram_tensor("m01", [PB, L], F32, kind="ExternalInput").ap()
    raw_out = nc.dram_tensor("raw", [PB, NCH], F32, kind="ExternalOutput").ap()
    ginv_out = nc.dram_tensor("ginv", [PB, T], BF16, kind="ExternalOutput").ap()
    with tile.TileContext(nc) as tcx:
        with ExitStack() as ctx:
            _emit(ctx, tcx, y_in, idxs_in, m01_in, raw_out, ginv_out)
    nc.compile()
    return nc


def _run(in_maps, **kwargs):
    if "nc" not in _CACHE:
        _CACHE["nc"] = _build()
    return run_bass_kernel_spmd(_CACHE["nc"], in_maps,
                                core_ids=list(range(NCORES)), **kwargs)


def kernel(y_true: np.ndarray, y_pred: np.ndarray, **run_kwargs) -> np.ndarray:
    assert y_pred.shape == (B, T, C), y_pred.shape
    in_maps = []
    for c in range(NCORES):
        sl = slice(c * PB, (c + 1) * PB)
        prep = _host_prep(y_true[sl])
        in_maps.append({"ypred": np.ascontiguousarray(y_pred[sl], np.float32),
                        "idxs": prep["idxs"], "m01": prep["m01"]})
    res = _run(in_maps, **run_kwargs)
    raw = np.concatenate([res.results[c]["raw"] for c in range(NCORES)], axis=0)
    ginv = np.concatenate([res.results[c]["ginv"] for c in range(NCORES)],
                          axis=0).astype(np.float64)
    lng = np.log(ginv).sum(axis=1)
    val = np.log(raw[:, 0].astype(np.float64))
    val += np.log(raw[:, 1:].astype(np.float64)).sum(axis=1)
    loss = -(val - lng)
    if run_kwargs:
        kernel.last_results = res  # expose trace info to test harness
    return loss[:, None].astype(np.float32)
